# revision 15
# baseline (speedup 1.0000x reference)
"""DecoderBlock (pre-norm attn + locality-bias softmax + pre-norm MLP) on 8 trn2 cores.

Sharding: batch-parallel (B=8 -> 1 batch element per core), no collectives.
On-device layout is fully transposed ([feature, token]) so all per-feature
scales/biases are per-partition scalars.

The locality bias lw*loc is rank-4:
  loc[i,j] = -(|ci|^2 + |cj|^2 - 2 ci.cj) / max_d2
so it is folded into 4 extra contraction dims appended to q/k.
exp(temperature[h]) is folded into w_q columns host-side.
ls1/ls2 fold into w_proj/w2; LN gain/bias applied on device (per-partition).
"""

import numpy as np
import ml_dtypes

B, N, C, H, D = 8, 1024, 768, 12, 64
GH, GW = 32, 32
HID = 3072
EPS = 1e-5
NC_ = C // 128          # 6 c-tiles
NH_ = HID // 128        # 24 hid-tiles
NT_ = N // 128          # 8 token-tiles
NCORES = 8

BF16 = ml_dtypes.bfloat16

_cached = {}


def _build_nc():
    import concourse.mybir as mybir
    import concourse.tile as tile
    from concourse import bacc

    dt = mybir.dt
    AF = mybir.ActivationFunctionType
    OP = mybir.AluOpType

    nc = bacc.Bacc(target_bir_lowering=False)

    # ---- DRAM IO ----
    xt_d = nc.declare_dram_parameter("xT", [128, NC_, N], dt.float32, isOutput=False)
    wqk_d = nc.declare_dram_parameter("wqk", [12, 128, NC_, 128], dt.bfloat16, isOutput=False)
    wv_d = nc.declare_dram_parameter("wv", [128, NC_, C], dt.bfloat16, isOutput=False)
    augq_d = nc.declare_dram_parameter("augq", [4, H, N], dt.bfloat16, isOutput=False)
    augk_d = nc.declare_dram_parameter("augk", [4, H, N], dt.bfloat16, isOutput=False)
    wproj_d = nc.declare_dram_parameter("wproj", [NC_, 128, NC_, 128], dt.bfloat16, isOutput=False)
    w1_d = nc.declare_dram_parameter("w1", [NH_, 128, NC_, 128], dt.bfloat16, isOutput=False)
    w2_d = nc.declare_dram_parameter("w2", [NC_, 128, NH_, 128], dt.bfloat16, isOutput=False)
    # misc consts packed: cols [0:24]=b1, [24:30]=bp, [30:36]=b2, [36:60]=ln g1,b1,g2,b2
    misc_d = nc.declare_dram_parameter("misc", [128, 60], dt.float32, isOutput=False)
    out_d = nc.declare_dram_parameter("outT", [128, NC_, N], dt.float32, isOutput=True)

    f32r = dt.float32r

    with tile.TileContext(nc) as tc:
        with (
            tc.tile_pool(name="consts", bufs=1) as consts,
            tc.tile_pool(name="resid", bufs=1) as resid,
            tc.tile_pool(name="rows", bufs=2) as rows,
            tc.tile_pool(name="bcast", bufs=1) as bcast,
            tc.tile_pool(name="scratch", bufs=3) as scratch,
        ):
            misc_t = consts.tile([128, 60], dt.float32, tag="misc")
            nc.sync.dma_start(out=misc_t, in_=misc_d[:])
            b1_t = misc_t[:, 0:24]
            bp_t = misc_t[:, 24:30]
            b2_t = misc_t[:, 30:36]
            g1_ap = misc_t[:, 36:42]
            bb1_ap = misc_t[:, 42:48]
            g2_ap = misc_t[:, 48:54]
            bb2_ap = misc_t[:, 54:60]
            ones_t = consts.tile([128, 1], dt.bfloat16, tag="onesb")
            nc.vector.memset(ones_t, 1.0)
            eps_c = consts.tile([1, 1], dt.float32, tag="epsc")
            nc.vector.memset(eps_c, EPS)
            eps_t = eps_c

            xt = resid.tile([128, NC_, N], dt.float32, tag="xt")
            nc.sync.dma_start(out=xt, in_=xt_d[:])

            def layer_norm_T(src, dst, g_ap, b_ap, psum_pool):
                """src: [128, NC_, N] f32 tile; dst: [128, NC_, N] bf16 tile.
                Stats over the partition(feature) axis via PE ones-matmuls."""
                mu_ps = psum_pool.tile([1, N], dt.float32, tag="stat_mu")
                sq_ps = psum_pool.tile([1, N], dt.float32, tag="stat_sq")
                for t in range(NC_):
                    xb = scratch.tile([128, N], dt.bfloat16, tag="scrb")
                    nc.vector.tensor_copy(xb, src[:, t, :])
                    sq = scratch.tile([128, N], dt.bfloat16, tag="scrb")
                    nc.vector.tensor_mul(sq, xb, xb)
                    for half in range(2):
                        sl = slice(half * 512, half * 512 + 512)
                        nc.tensor.matmul(
                            mu_ps[:, sl], ones_t, xb[:, sl],
                            start=(t == 0), stop=(t == NC_ - 1),
                        )
                        nc.tensor.matmul(
                            sq_ps[:, sl], ones_t, sq[:, sl],
                            start=(t == 0), stop=(t == NC_ - 1),
                        )
                mu_row = rows.tile([1, N], dt.float32, tag="r_mu")
                ms_row = rows.tile([1, N], dt.float32, tag="r_msden")
                nc.scalar.activation(mu_row, mu_ps, AF.Copy, scale=1.0 / C)
                nc.scalar.activation(ms_row, sq_ps, AF.Copy, scale=1.0 / C)
                std_row = rows.tile([1, N], dt.float32, tag="r_std")
                nc.vector.tensor_mul(std_row, mu_row, mu_row)
                nc.vector.tensor_sub(std_row, ms_row, std_row)
                nc.scalar.activation(std_row, std_row, AF.Sqrt, bias=eps_t)
                nc.vector.reciprocal(std_row, std_row)
                mu_b = bcast.tile([128, N], dt.float32, tag="mu_b")
                rstd_b = bcast.tile([128, N], dt.float32, tag="rstd_b")
                nc.gpsimd.partition_broadcast(mu_b, mu_row)
                nc.gpsimd.partition_broadcast(rstd_b, std_row)
                for t in range(NC_):
                    t1 = scratch.tile([128, N], dt.float32, tag="scr")
                    nc.vector.tensor_sub(t1, src[:, t, :], mu_b)
                    nc.vector.tensor_mul(t1, t1, rstd_b)
                    nc.vector.tensor_scalar(
                        out=dst[:, t, :], in0=t1,
                        scalar1=g_ap[:, t : t + 1], scalar2=b_ap[:, t : t + 1],
                        op0=OP.mult, op1=OP.add,
                    )

            # attnT opens first so it can outlive the q/k/v pools (LIFO scoping)
            with tc.tile_pool(name="attnT_pool", bufs=1) as attp:
                attnT = attp.tile([128, NC_, N], dt.bfloat16, tag="attnT")

                with tc.tile_pool(name="qkv_acts", bufs=1) as qkvp:
                    qt = qkvp.tile([68, H, N], dt.bfloat16, tag="qt")
                    kt = qkvp.tile([68, H, N], dt.bfloat16, tag="kt")
                    vaug = qkvp.tile([128, H, NT_, 65], dt.bfloat16, tag="vaug")
                    nc.sync.dma_start(out=qt[64:68, :, :], in_=augq_d[:])
                    nc.sync.dma_start(out=kt[64:68, :, :], in_=augk_d[:])
                    nc.vector.memset(vaug[:, :, :, 64:65], 1.0)

                    with tc.tile_pool(name="n1_pool", bufs=1) as n1p:
                        n1 = n1p.tile([128, NC_, N], dt.bfloat16, tag="n1")
                        # -------- Phase 0: LN1 --------
                        with tc.tile_pool(name="psum_ln1", bufs=1, space="PSUM") as pp:
                            layer_norm_T(xt, n1, g1_ap, bb1_ap, pp)

                        # -------- Phase V: v generation (row layout) --------
                        with (
                            tc.tile_pool(name="wv_pool", bufs=1) as wvp,
                            tc.tile_pool(name="psum_v", bufs=2, space="PSUM") as pp,
                        ):
                            wv_sb = wvp.tile([128, NC_, C], dt.bfloat16, tag="wv")
                            nc.sync.dma_start(out=wv_sb, in_=wv_d[:])
                            for i in range(NT_):
                                ps = pp.tile([128, C], dt.float32, tag="v_ps")
                                for sl in (slice(0, 512), slice(512, 768)):
                                    for k in range(NC_):
                                        nc.tensor.matmul(
                                            ps[:, sl],
                                            n1[:, k, i * 128 : i * 128 + 128],
                                            wv_sb[:, k, sl],
                                            start=(k == 0), stop=(k == NC_ - 1),
                                        )
                                nc.scalar.copy(
                                    out=vaug[:, :, i, 0:64],
                                    in_=ps[:].rearrange("p (h d) -> p h d", h=H),
                                )

                        # -------- Phase QK: q/k generation (transposed) --------
                        with (
                            tc.tile_pool(name="wqk_pool", bufs=3) as wpool,
                            tc.tile_pool(name="psum_qk", bufs=2, space="PSUM") as pp,
                        ):
                            for m in range(12):
                                w_sb = wpool.tile([128, NC_, 128], dt.bfloat16, tag="wqk")
                                nc.sync.dma_start(out=w_sb, in_=wqk_d[m])
                                ps = pp.tile([128, N], dt.float32, tag="qk_ps")
                                for half in range(2):
                                    sl = slice(half * 512, half * 512 + 512)
                                    for k in range(NC_):
                                        nc.tensor.matmul(
                                            ps[:, sl], w_sb[:, k, :], n1[:, k, sl],
                                            start=(k == 0), stop=(k == NC_ - 1),
                                        )
                                dst = qt if m < 6 else kt
                                h0 = 2 * (m % 6)
                                nc.scalar.copy(out=dst[0:64, h0, :], in_=ps[0:64, :])
                                nc.scalar.copy(out=dst[0:64, h0 + 1, :], in_=ps[64:128, :])

                    # -------- Phase A: attention --------
                    with (
                        tc.tile_pool(name="pt_pool", bufs=3) as ptp,
                        tc.tile_pool(name="rec_pool", bufs=2) as recp,
                        tc.tile_pool(name="psum_s", bufs=2, space="PSUM") as pps,
                        tc.tile_pool(name="psum_av", bufs=2, space="PSUM") as ppav,
                    ):
                        for h in range(H):
                            av = ppav.tile([65, N], dt.float32, tag="av")
                            for j in range(NT_):
                                s_ps = pps.tile([128, N], dt.float32, tag="s")
                                for half in range(2):
                                    sl = slice(half * 512, half * 512 + 512)
                                    nc.tensor.matmul(
                                        s_ps[:, sl],
                                        kt[:, h, j * 128 : j * 128 + 128],
                                        qt[:, h, sl],
                                        start=True, stop=True,
                                    )
                                pt = ptp.tile([128, N], dt.bfloat16, tag="pt")
                                nc.scalar.activation(pt, s_ps, AF.Exp)
                                for half in range(2):
                                    sl = slice(half * 512, half * 512 + 512)
                                    nc.tensor.matmul(
                                        av[:, sl], vaug[:, h, j, :], pt[:, sl],
                                        start=(j == 0), stop=(j == NT_ - 1),
                                    )
                            den = rows.tile([1, N], dt.float32, tag="r_msden")
                            nc.scalar.copy(out=den, in_=av[64:65, :])
                            nc.vector.reciprocal(den, den)
                            rec_b = recp.tile([64, N], dt.float32, tag="rec_b")
                            nc.gpsimd.partition_broadcast(rec_b, den)
                            r0 = (h % 2) * 64
                            nc.vector.tensor_mul(
                                attnT[r0 : r0 + 64, h // 2, :], av[0:64, :], rec_b
                            )

                # -------- Phase P: proj + residual --------
                x1 = resid.tile([128, NC_, N], dt.float32, tag="x1")
                with (
                    tc.tile_pool(name="wp_pool", bufs=2) as wpp,
                    tc.tile_pool(name="psum_p", bufs=2, space="PSUM") as pp,
                ):
                    for m in range(NC_):
                        w_sb = wpp.tile([128, NC_, 128], dt.bfloat16, tag="wp")
                        nc.sync.dma_start(out=w_sb, in_=wproj_d[m])
                        ps = pp.tile([128, N], dt.float32, tag="p_ps")
                        for half in range(2):
                            sl = slice(half * 512, half * 512 + 512)
                            for k in range(NC_):
                                nc.tensor.matmul(
                                    ps[:, sl], w_sb[:, k, :], attnT[:, k, sl],
                                    start=(k == 0), stop=(k == NC_ - 1),
                                )
                        nc.vector.scalar_tensor_tensor(
                            out=x1[:, m, :], in0=ps, scalar=bp_t[:, m : m + 1],
                            in1=xt[:, m, :], op0=OP.add, op1=OP.add,
                        )

            # -------- Phase 5: LN2 + MLP --------
            with tc.tile_pool(name="mlp_acts", bufs=1) as mlpp:
                n2 = mlpp.tile([128, NC_, N], dt.bfloat16, tag="n2")
                g1 = mlpp.tile([128, NH_, N], dt.bfloat16, tag="g1")
                with tc.tile_pool(name="psum_ln2", bufs=1, space="PSUM") as pp:
                    layer_norm_T(x1, n2, g2_ap, bb2_ap, pp)

                with (
                    tc.tile_pool(name="w1_pool", bufs=3) as w1p,
                    tc.tile_pool(name="psum_f1", bufs=2, space="PSUM") as pp,
                ):
                    for m in range(NH_):
                        w_sb = w1p.tile([128, NC_, 128], dt.bfloat16, tag="w1")
                        nc.sync.dma_start(out=w_sb, in_=w1_d[m])
                        ps = pp.tile([128, N], dt.float32, tag="f1_ps")
                        for half in range(2):
                            sl = slice(half * 512, half * 512 + 512)
                            for k in range(NC_):
                                nc.tensor.matmul(
                                    ps[:, sl], w_sb[:, k, :], n2[:, k, sl],
                                    start=(k == 0), stop=(k == NC_ - 1),
                                )
                        nc.scalar.activation(
                            g1[:, m, :], ps, AF.Gelu, bias=b1_t[:, m : m + 1]
                        )

                with (
                    tc.tile_pool(name="w2_pool", bufs=2) as w2p,
                    tc.tile_pool(name="out_pool", bufs=2) as outp,
                    tc.tile_pool(name="psum_f2", bufs=2, space="PSUM") as pp,
                ):
                    for m in range(NC_):
                        w_sb = w2p.tile([128, NH_, 128], dt.bfloat16, tag="w2")
                        nc.sync.dma_start(out=w_sb, in_=w2_d[m])
                        ps = pp.tile([128, N], dt.float32, tag="f2_ps")
                        for half in range(2):
                            sl = slice(half * 512, half * 512 + 512)
                            for k in range(NH_):
                                nc.tensor.matmul(
                                    ps[:, sl], w_sb[:, k, :], g1[:, k, sl],
                                    start=(k == 0), stop=(k == NH_ - 1),
                                )
                        ot = outp.tile([128, N], dt.float32, tag="ot")
                        nc.vector.scalar_tensor_tensor(
                            out=ot, in0=ps, scalar=b2_t[:, m : m + 1],
                            in1=x1[:, m, :], op0=OP.add, op1=OP.add,
                        )
                        nc.sync.dma_start(out=out_d[:, m, :], in_=ot)

    nc.finalize()  # Bacc.compile(): wait-splitting, library/ACT-table loads, ISA bytes
    return nc


def _prep_inputs(x, w_qkv, w_proj, b_proj, ln1_g, ln1_b, ln2_g, ln2_b,
                 temperature, locality_weight, ls1, ls2, w1, b1, w2, b2):
    """Host-side folds + tiling. Returns (shared_map, per_core_xts)."""
    f32 = np.float32
    scale = np.exp(np.asarray(temperature, np.float64)).reshape(H).astype(f32)
    lw = np.asarray(locality_weight, f32)

    wq = np.array(w_qkv[:, 0:C], f32, copy=True)
    for h in range(H):
        wq[:, h * D : (h + 1) * D] *= scale[h]
    wk = np.asarray(w_qkv[:, C : 2 * C], f32)
    wv = np.asarray(w_qkv[:, 2 * C : 3 * C], f32)

    wqk = np.concatenate([wq, wk], axis=1)  # [768, 1536]
    wqk_t = np.ascontiguousarray(
        wqk.reshape(NC_, 128, 12, 128).transpose(2, 1, 0, 3)
    ).astype(BF16)
    wv_t = np.ascontiguousarray(
        wv.reshape(NC_, 128, C).transpose(1, 0, 2)).astype(BF16)

    # locality bias aug factors
    gy, gx = np.meshgrid(np.linspace(0.0, 1.0, GH), np.linspace(0.0, 1.0, GW),
                         indexing="ij")
    coords = np.stack([gy.ravel(), gx.ravel()], axis=-1).astype(np.float64)  # [N,2]
    d2 = ((coords[None, :, :] - coords[:, None, :]) ** 2).sum(-1)
    M = d2.max()
    nrm2 = (coords ** 2).sum(-1)  # |c_i|^2
    augq = np.zeros((4, H, N), np.float64)
    augk = np.zeros((4, H, N), np.float64)
    for h in range(H):
        l = float(lw[h])
        s = np.sqrt(2.0 * abs(l) / M)
        sgn = 1.0 if l >= 0 else -1.0
        augq[0, h] = -(l / M) * nrm2
        augq[1, h] = 1.0
        augq[2, h] = s * coords[:, 0]
        augq[3, h] = s * coords[:, 1]
        augk[0, h] = 1.0
        augk[1, h] = -(l / M) * nrm2
        augk[2, h] = sgn * s * coords[:, 0]
        augk[3, h] = sgn * s * coords[:, 1]
    augq = augq.astype(BF16)
    augk = augk.astype(BF16)

    wproj_t = np.ascontiguousarray(
        (np.asarray(w_proj, f32) * np.asarray(ls1, f32)[None, :])
        .reshape(NC_, 128, NC_, 128).transpose(2, 1, 0, 3)
    ).astype(BF16)
    w1_t = np.ascontiguousarray(
        np.asarray(w1, f32).reshape(NC_, 128, NH_, 128).transpose(2, 1, 0, 3)
    ).astype(BF16)
    w2_t = np.ascontiguousarray(
        (np.asarray(w2, f32) * np.asarray(ls2, f32)[None, :])
        .reshape(NH_, 128, NC_, 128).transpose(2, 1, 0, 3)
    ).astype(BF16)

    def colmaj(v):
        return np.asarray(v, f32).reshape(-1, 128).T  # [128, ntiles]

    misc = np.concatenate([
        colmaj(np.asarray(b1, f32)),                       # [128, 24]
        colmaj(np.asarray(b_proj, f32) * np.asarray(ls1, f32)),   # [128, 6]
        colmaj(np.asarray(b2, f32) * np.asarray(ls2, f32)),       # [128, 6]
        colmaj(ln1_g), colmaj(ln1_b), colmaj(ln2_g), colmaj(ln2_b),  # 4x[128,6]
    ], axis=1)
    misc = np.ascontiguousarray(misc).astype(f32)

    shared = {
        "wqk": wqk_t, "wv": wv_t, "augq": augq, "augk": augk,
        "wproj": wproj_t, "w1": w1_t, "w2": w2_t, "misc": misc,
    }
    xts = []
    for b in range(B):
        xT = np.asarray(x[b], f32).T  # [768, 1024]
        xts.append(np.ascontiguousarray(
            xT.reshape(NC_, 128, N).transpose(1, 0, 2)).astype(f32))
    return shared, xts


def kernel(**inputs):
    from concourse import bass_utils

    if "nc" not in _cached:
        _cached["nc"] = _build_nc()
    nc = _cached["nc"]

    shared, xts = _prep_inputs(**inputs)
    in_maps = [{**shared, "xT": xts[b]} for b in range(NCORES)]
    res = bass_utils.run_bass_kernel_spmd(nc, in_maps, core_ids=list(range(NCORES)))
    outs = []
    for b in range(NCORES):
        oT = res.results[b]["outT"]  # [128, NC_, N]
        outs.append(np.ascontiguousarray(
            oT.transpose(1, 0, 2).reshape(C, N).T))
    return np.stack(outs).astype(np.float32)
